# revision 6
# baseline (speedup 1.0000x reference)
"""CrissCrossAttention Trainium2 kernel.

Data-parallel over batch: 8 images -> 8 NeuronCores, one image per core.

Per-core algorithm (C=512, H=W=96, D=CQK=64, S=H*W=9216):
  Pass 0: q = WqT.T @ x + bq, k likewise  (kept in SBUF, bf16, [64, S])
          vt[s, c] = (Wv @ x + bv).T      (spatial-major v, spilled to DRAM bf16)
  Phase 1 (per column w): eHT[g,h] = Kw.T @ Qw; diag-mask; ee = exp(e-40) bf16
          outH_raw[c, h] = vt_col_w.T @ ee  (4 c-chunks);  Z_H[h,w] = ee.T @ 1
          OUT[c, :, w] = outH_raw
  Phase 2 (per row h): eWT[t,w] = Kh.T @ Qh; ee2 = exp(e-40)
          OUT[c, h, :] += vt_row_h.T @ ee2;  Z_W[w,h] = ee2.T @ 1
  r' = gamma / (Z_H + Z_W.T)   (exp shift cancels between numerator and Z)
  out = OUT * r' + x           (bv is folded into vt; softmax rows sum to 1)

exp is computed without per-row max subtraction: energies for these inputs
are bounded well inside exp's f32 range; a constant -40 shift guards the
high side and cancels exactly in the normalization.
"""

import os
import sys

import numpy as np

for _p in ("/opt/trn_rl_repo",):
    if os.path.isdir(_p) and _p not in sys.path:
        sys.path.insert(0, _p)

import ml_dtypes  # noqa: E402

BF16 = ml_dtypes.bfloat16

B, C, HP, WP = 8, 512, 96, 96
S = HP * WP
D = 64
KO = C // 128
NT = S // 512  # spatial tiles in pass 0 / final
N_CORES = 8

_cache = {}


def _build_nc():
    import concourse.bass as bass
    import concourse.bacc as bacc
    import concourse.mybir as mybir
    import concourse.tile as tile
    from concourse.bass import ts, ds

    f32 = mybir.dt.float32
    bf16 = mybir.dt.bfloat16
    ADD = mybir.AluOpType.add
    MULT = mybir.AluOpType.mult
    EXP = mybir.ActivationFunctionType.Exp
    IDENT = mybir.ActivationFunctionType.Identity

    nc = bacc.Bacc()

    x = nc.declare_dram_parameter("x", [KO, 128, S], f32, isOutput=False)
    wqT = nc.declare_dram_parameter("wqT", [KO, 128, D], bf16, isOutput=False)
    wkT = nc.declare_dram_parameter("wkT", [KO, 128, D], bf16, isOutput=False)
    wvT = nc.declare_dram_parameter("wvT", [KO, 128, C], bf16, isOutput=False)
    bq = nc.declare_dram_parameter("bq", [D, 1], f32, isOutput=False)
    bk = nc.declare_dram_parameter("bk", [D, 1], f32, isOutput=False)
    bv = nc.declare_dram_parameter("bv", [1, C], f32, isOutput=False)
    gamma = nc.declare_dram_parameter("gamma", [1, 1], f32, isOutput=False)
    diagneg = nc.declare_dram_parameter("diagneg", [HP, HP], f32, isOutput=False)
    id96 = nc.declare_dram_parameter("id96", [HP, HP], f32, isOutput=False)
    ones96 = nc.declare_dram_parameter("ones96", [HP, 1], bf16, isOutput=False)
    out = nc.declare_dram_parameter("out", [KO, 128, S], f32, isOutput=True)

    vt_dram = nc.dram_tensor("vt_spill", [S, C], bf16)
    r_dram = nc.dram_tensor("r_bounce", [1, S], bf16)

    x_ap = x[:, :, :]
    x_r = x_ap.rearrange("ko ki s -> ki ko s")
    out_ap = out[:, :, :]
    vt_ap = vt_dram[:, :]
    # column view of vt: s = g*WP + w  ->  [w][g, c]
    vt_col = vt_ap.rearrange("(g w) c -> w g c", w=WP)
    r_ap = r_dram[:, :]

    with tile.TileContext(nc) as tc:
        with tc.tile_pool(name="consts", bufs=1) as consts:
            wq_sb = consts.tile([128, KO, D], bf16)
            wk_sb = consts.tile([128, KO, D], bf16)
            wv_sb = consts.tile([128, KO, C], bf16)
            for ko in range(KO):
                nc.sync.dma_start(wq_sb[:, ko, :], wqT[ko, :, :])
                nc.sync.dma_start(wk_sb[:, ko, :], wkT[ko, :, :])
                nc.sync.dma_start(wv_sb[:, ko, :], wvT[ko, :, :])
            bq_sb = consts.tile([D, 1], f32)
            bk_sb = consts.tile([D, 1], f32)
            nc.sync.dma_start(bq_sb[:], bq[:, :])
            nc.sync.dma_start(bk_sb[:], bk[:, :])
            bv_sb = consts.tile([128, C], f32)
            nc.sync.dma_start(bv_sb[:], bv[:, :].to_broadcast((128, C)))
            gam_sb = consts.tile([HP, 1], f32)
            nc.sync.dma_start(gam_sb[:], gamma[:, :].to_broadcast((HP, 1)))
            dneg_sb = consts.tile([HP, HP], f32)
            nc.sync.dma_start(dneg_sb[:], diagneg[:, :])
            id_sb = consts.tile([HP, HP], f32)
            nc.sync.dma_start(id_sb[:], id96[:, :])
            ones_sb = consts.tile([HP, 1], bf16)
            nc.sync.dma_start(ones_sb[:], ones96[:, :])
            shift_sb = consts.tile([HP, 1], f32)
            nc.vector.memset(shift_sb[:], -40.0)

            q_sb = consts.tile([D, S], bf16)
            k_sb = consts.tile([D, S], bf16)
            OUTB = consts.tile([128, KO, S], bf16)
            ZH = consts.tile([HP, HP], f32)
            ZW = consts.tile([HP, HP], f32)

            # ---------------- Pass 0: projections ----------------
            with (
                tc.tile_pool(name="xio", bufs=2) as xio,
                tc.tile_pool(name="vtio", bufs=3) as vtio,
                tc.tile_pool(name="ps0", bufs=2, space="PSUM") as ps0,
            ):
                for it in range(NT):
                    xt = xio.tile([128, KO, 512], f32, tag="xt")
                    nc.gpsimd.dma_start(xt[:], x_r[:, :, ts(it, 512)])
                    xb = xio.tile([128, KO, 512], bf16, tag="xb")
                    nc.vector.tensor_copy(xb[:], xt[:])

                    qp = ps0.tile([D, 512], f32, tag="qp")
                    for ko in range(KO):
                        nc.tensor.matmul(
                            qp[:], wq_sb[:, ko, :], xb[:, ko, :],
                            start=(ko == 0), stop=(ko == KO - 1),
                        )
                    kp = ps0.tile([D, 512], f32, tag="kp")
                    for ko in range(KO):
                        nc.tensor.matmul(
                            kp[:], wk_sb[:, ko, :], xb[:, ko, :],
                            start=(ko == 0), stop=(ko == KO - 1),
                        )
                    nc.scalar.activation(q_sb[:, ts(it, 512)], qp[:], IDENT, bias=bq_sb[:])
                    nc.scalar.activation(k_sb[:, ts(it, 512)], kp[:], IDENT, bias=bk_sb[:])

                    for j in range(4):
                        vp = ps0.tile([128, C], f32, tag="vp")
                        for ko in range(KO):
                            nc.tensor.matmul(
                                vp[:], xb[:, ko, ts(j, 128)], wv_sb[:, ko, :],
                                start=(ko == 0), stop=(ko == KO - 1),
                            )
                        vtt = vtio.tile([128, C], bf16, tag="vtt")
                        nc.vector.tensor_tensor(vtt[:], vp[:], bv_sb[:], ADD)
                        nc.gpsimd.dma_start(
                            vt_ap[ds(it * 512 + j * 128, 128), :], vtt[:]
                        )

            # column/row views of q, k: s = g*WP + w
            q_colv = q_sb[:, :].rearrange("d (g w) -> w d g", w=WP)
            k_colv = k_sb[:, :].rearrange("d (g w) -> w d g", w=WP)
            OUT_colv = OUTB[:, :, :].rearrange("p ko (g w) -> w p ko g", w=WP)

            # ---------------- Phases 1 & 2: attention ----------------
            with (
                tc.tile_pool(name="vtio2", bufs=3) as vtio2,
                tc.tile_pool(name="attw", bufs=3) as attw,
                tc.tile_pool(name="psA", bufs=2, space="PSUM") as psA,
            ):
                # Phase 1: column (height-axis) attention, per w
                for w in range(WP):
                    vtc = vtio2.tile([HP, C], bf16, tag="vtc")
                    nc.gpsimd.dma_start(vtc[:], vt_col[w, :, :])
                    ep = psA.tile([HP, HP], f32, tag="ep")
                    nc.tensor.matmul(ep[:], k_colv[w, :, :], q_colv[w, :, :],
                                     start=True, stop=True)
                    nc.vector.tensor_tensor(ep[:], ep[:], dneg_sb[:], ADD)
                    ee = attw.tile([HP, HP], bf16, tag="ee")
                    nc.scalar.activation(ee[:], ep[:], EXP)
                    op = psA.tile([128, KO, HP], f32, tag="op")
                    for cc in range(KO):
                        nc.tensor.matmul(op[:, cc, :], vtc[:, ts(cc, 128)], ee[:],
                                         start=True, stop=True)
                    zp = psA.tile([HP, 1], f32, tag="zp")
                    nc.tensor.matmul(zp[:], ee[:], ones_sb[:], start=True, stop=True)
                    nc.scalar.copy(ZH[:, ds(w, 1)], zp[:])
                    nc.vector.tensor_copy(OUT_colv[w, :, :, :], op[:])

                # Phase 2: row (width-axis) attention, per h
                for h in range(HP):
                    vtr = vtio2.tile([HP, C], bf16, tag="vtc")
                    nc.gpsimd.dma_start(vtr[:], vt_ap[ds(h * WP, WP), :])
                    ep2 = psA.tile([HP, HP], f32, tag="ep")
                    nc.tensor.matmul(ep2[:], k_sb[:, ds(h * WP, WP)],
                                     q_sb[:, ds(h * WP, WP)], start=True, stop=True)
                    ee2 = attw.tile([HP, HP], bf16, tag="ee")
                    nc.scalar.activation(ee2[:], ep2[:], EXP, bias=shift_sb[:])
                    op2 = psA.tile([128, KO, HP], f32, tag="op")
                    for cc in range(KO):
                        nc.tensor.matmul(op2[:, cc, :], vtr[:, ts(cc, 128)], ee2[:],
                                         start=True, stop=True)
                    zp2 = psA.tile([HP, 1], f32, tag="zp")
                    nc.tensor.matmul(zp2[:], ee2[:], ones_sb[:], start=True, stop=True)
                    nc.scalar.copy(ZW[:, ds(h, 1)], zp2[:])
                    outsl = OUTB[:, :, ds(h * WP, WP)]
                    nc.vector.tensor_tensor(outsl, op2[:], outsl, ADD)

                # ---------------- normalization map ----------------
                ztp = psA.tile([HP, HP], f32, tag="ep")
                nc.tensor.transpose(ztp[:], ZW[:], id_sb[:])
                zs = consts.tile([HP, HP], f32)
                nc.vector.tensor_tensor(zs[:], ztp[:], ZH[:], ADD)
                rm = consts.tile([HP, HP], f32)
                nc.vector.reciprocal(rm[:], zs[:])
                nc.vector.tensor_scalar_mul(rm[:], rm[:], gam_sb[:])
                rmb = consts.tile([HP, HP], bf16)
                nc.vector.tensor_copy(rmb[:], rm[:])
                nc.sync.dma_start(
                    r_ap.rearrange("a (h w) -> (a h) w", h=HP), rmb[:]
                )

            rb = consts.tile([128, S], bf16)
            nc.sync.dma_start(rb[:], r_ap.to_broadcast((128, S)))

            # ---------------- final: out = OUT * r' + x ----------------
            with tc.tile_pool(name="fin", bufs=3) as fin:
                for it in range(NT):
                    for ko in range(KO):
                        xt2 = fin.tile([128, 512], f32, tag="xt2")
                        nc.gpsimd.dma_start(xt2[:], x_ap[ko, :, ts(it, 512)])
                        t1 = fin.tile([128, 512], f32, tag="t1")
                        nc.vector.tensor_tensor(
                            t1[:], OUTB[:, ko, ts(it, 512)], rb[:, ts(it, 512)], MULT
                        )
                        nc.vector.tensor_tensor(t1[:], t1[:], xt2[:], ADD)
                        nc.gpsimd.dma_start(out_ap[ko, :, ts(it, 512)], t1[:])

    nc.finalize()
    return nc


def _prep_in_maps(inputs):
    x = np.ascontiguousarray(np.asarray(inputs["x"]), dtype=np.float32)
    Wq = np.asarray(inputs["Wq"], dtype=np.float32)
    Wk = np.asarray(inputs["Wk"], dtype=np.float32)
    Wv = np.asarray(inputs["Wv"], dtype=np.float32)
    wqT = np.ascontiguousarray(Wq.T).astype(BF16).reshape(KO, 128, D)
    wkT = np.ascontiguousarray(Wk.T).astype(BF16).reshape(KO, 128, D)
    wvT = np.ascontiguousarray(Wv.T).astype(BF16).reshape(KO, 128, C)
    bq = np.asarray(inputs["bq"], dtype=np.float32).reshape(D, 1)
    bk = np.asarray(inputs["bk"], dtype=np.float32).reshape(D, 1)
    bv = np.asarray(inputs["bv"], dtype=np.float32).reshape(1, C)
    gamma = np.asarray(inputs["gamma"], dtype=np.float32).reshape(1, 1)
    diagneg = np.where(np.eye(HP, dtype=bool), np.float32(-1e30), np.float32(-40.0))
    diagneg = diagneg.astype(np.float32)
    id96 = np.eye(HP, dtype=np.float32)
    ones96 = np.ones((HP, 1), BF16)
    shared = dict(wqT=wqT, wkT=wkT, wvT=wvT, bq=bq, bk=bk, bv=bv,
                  gamma=gamma, diagneg=diagneg, id96=id96, ones96=ones96)
    in_maps = []
    for i in range(N_CORES):
        m = dict(shared)
        m["x"] = np.ascontiguousarray(x[i].reshape(KO, 128, S))
        in_maps.append(m)
    return in_maps


def kernel(**inputs) -> np.ndarray:
    from concourse.bass_utils import run_bass_kernel_spmd

    if "nc" not in _cache:
        _cache["nc"] = _build_nc()
    nc = _cache["nc"]

    in_maps = _prep_in_maps(inputs)
    trace = bool(int(os.environ.get("CC_TRACE", "0")))
    res = run_bass_kernel_spmd(
        nc, in_maps, core_ids=list(range(N_CORES)), trace=trace
    )
    _cache["last_result"] = res
    out = np.stack(
        [np.asarray(res.results[i]["out"]).reshape(C, HP, WP) for i in range(N_CORES)]
    )
    return out


# revision 34
# speedup vs baseline: 1.2779x; 1.2779x over previous
"""CrissCrossAttention Trainium2 kernel.

Data-parallel over batch: 8 images -> 8 NeuronCores, one image per core.

Per-core algorithm (C=512, H=W=96, D=CQK=64, S=H*W=9216):
  Pass 0: q = WqT.T @ x + bq, k likewise  (kept in SBUF, bf16, [64, S])
          vt[s, c] = (Wv @ x + bv).T      (spatial-major v, spilled to DRAM bf16)
  Phase 1 (per column w): eHT[g,h] = Kw.T @ Qw; diag-mask; ee = exp(e-40) bf16
          outH_raw[c, h] = vt_col_w.T @ ee  (4 c-chunks);  Z_H[h,w] = ee.T @ 1
          OUT[c, :, w] = outH_raw
  Phase 2 (per row h): eWT[t,w] = Kh.T @ Qh; ee2 = exp(e-40)
          OUT[c, h, :] += vt_row_h.T @ ee2;  Z_W[w,h] = ee2.T @ 1
  r' = gamma / (Z_H + Z_W.T)   (exp shift cancels between numerator and Z)
  out = OUT * r' + x           (bv is folded into vt; softmax rows sum to 1)

exp is computed without per-row max subtraction: energies for these inputs
are bounded well inside exp's f32 range; a constant -40 shift guards the
high side and cancels exactly in the normalization.
"""

import os
import sys

import numpy as np

for _p in ("/opt/trn_rl_repo",):
    if os.path.isdir(_p) and _p not in sys.path:
        sys.path.insert(0, _p)

import ml_dtypes  # noqa: E402

BF16 = ml_dtypes.bfloat16

B, C, HP, WP = 8, 512, 96, 96
S = HP * WP
D = 64
KO = C // 128
NT = S // 512  # spatial tiles in pass 0 / final
QB = 2  # columns/rows per phase iteration
N_CORES = 8

_cache = {}


def _build_nc(phases=(0, 1, 2, 3), xio_bufs=3, ps0_bufs=2, psA_bufs=2, vtio_bufs=3, vtio2_bufs=7, attw_bufs=4, fin_bufs=3, xpre_bufs=4):
    import concourse.bass as bass
    import concourse.bacc as bacc
    import concourse.mybir as mybir
    import concourse.tile as tile
    from concourse.bass import ts, ds

    f32 = mybir.dt.float32
    bf16 = mybir.dt.bfloat16
    ADD = mybir.AluOpType.add
    MULT = mybir.AluOpType.mult
    EXP = mybir.ActivationFunctionType.Exp
    IDENT = mybir.ActivationFunctionType.Identity

    nc = bacc.Bacc()

    x = nc.declare_dram_parameter("x", [KO, 128, S], f32, isOutput=False)
    wqkT = nc.declare_dram_parameter("wqkT", [KO, 128, 2 * D], bf16, isOutput=False)
    wvT = nc.declare_dram_parameter("wvT", [KO, 128, C], bf16, isOutput=False)
    bq = nc.declare_dram_parameter("bq", [D, 1], f32, isOutput=False)
    bk = nc.declare_dram_parameter("bk", [D, 1], f32, isOutput=False)
    bv = nc.declare_dram_parameter("bv", [1, C], f32, isOutput=False)
    gamma = nc.declare_dram_parameter("gamma", [1, 1], f32, isOutput=False)
    id96 = nc.declare_dram_parameter("id96", [HP, HP], f32, isOutput=False)
    negeye = nc.declare_dram_parameter("negeye", [HP, HP], bf16, isOutput=False)
    eyeb = nc.declare_dram_parameter("eyeb", [HP, HP], bf16, isOutput=False)
    ones96 = nc.declare_dram_parameter("ones96", [HP, 1], bf16, isOutput=False)
    out = nc.declare_dram_parameter("out", [KO, 128, S], f32, isOutput=True)

    fp8 = mybir.dt.float8e4
    vt_dram = nc.dram_tensor("vt_spill", [S, C], fp8)
    r_dram = nc.dram_tensor("r_bounce", [1, S], bf16)

    x_ap = x[:, :, :]
    x_r = x_ap.rearrange("ko ki s -> ki ko s")
    out_ap = out[:, :, :]
    out_r = out_ap.rearrange("ko ki s -> ki ko s")
    vt_ap = vt_dram[:, :]
    # column view of vt: s = g*WP + w  ->  [w][g, c]
    vt_col = vt_ap.rearrange("(g w) c -> w g c", w=WP)
    r_ap = r_dram[:, :]

    with tile.TileContext(nc) as tc:
        with tc.tile_pool(name="consts", bufs=1) as consts:
            wqk_sb = consts.tile([128, KO, 2 * D], bf16)
            wv_sb = consts.tile([128, KO, C], bf16)
            for ko in range(KO):
                nc.sync.dma_start(wqk_sb[:, ko, :], wqkT[ko, :, :])
                nc.sync.dma_start(wv_sb[:, ko, :], wvT[ko, :, :])
            bq_sb = consts.tile([D, 1], f32)
            bk_sb = consts.tile([D, 1], f32)
            nc.sync.dma_start(bq_sb[:], bq[:, :])
            nc.sync.dma_start(bk_sb[:], bk[:, :])
            bv_sb = consts.tile([128, C], f32)
            nc.sync.dma_start(bv_sb[:], bv[:, :].to_broadcast((128, C)))
            gam_sb = consts.tile([HP, 1], f32)
            nc.sync.dma_start(gam_sb[:], gamma[:, :].to_broadcast((HP, 1)))
            id_sb = consts.tile([HP, HP], f32)
            nc.sync.dma_start(id_sb[:], id96[:, :])
            ones_sb = consts.tile([HP, 1], bf16)
            nc.sync.dma_start(ones_sb[:], ones96[:, :])
            negi_sb = consts.tile([HP, HP], bf16)
            nc.sync.dma_start(negi_sb[:], negeye[:, :])
            eyeb_sb = consts.tile([HP, HP], bf16)
            nc.sync.dma_start(eyeb_sb[:], eyeb[:, :])
            shift_sb = consts.tile([HP, 1], f32)
            nc.vector.memset(shift_sb[:], -40.0)

            qk_cm = tc.tile_pool(name="qk", bufs=1, side="right")
            qk_pool = qk_cm.__enter__()
            q_sb = qk_pool.tile([D, S], bf16)
            k_sb = qk_pool.tile([D, S], bf16)
            ZH = consts.tile([HP, HP], f32)
            ZW = consts.tile([HP, HP], f32)

            # ---------------- Pass 0: projections ----------------
            with (
                tc.tile_pool(name="xio", bufs=xio_bufs) as xio,
                tc.tile_pool(name="vtio", bufs=vtio_bufs) as vtio,
                tc.tile_pool(name="ps0", bufs=ps0_bufs, space="PSUM") as ps0,
            ):
                for it in range(NT):
                    xt = xio.tile([128, KO, 512], f32, tag="xt")
                    nc.gpsimd.dma_start(xt[:], x_r[:, :, ts(it, 512)])
                    xb = xio.tile([128, KO, 512], bf16, tag="xb")
                    for ko in range(KO):
                        nc.scalar.copy(xb[:, ko, :], xt[:, ko, :])

                    qkp = ps0.tile([2 * D, 512], f32, tag="qkp")
                    for ko in range(KO):
                        nc.tensor.matmul(
                            qkp[:], wqk_sb[:, ko, :], xb[:, ko, :],
                            start=(ko == 0), stop=(ko == KO - 1),
                        )
                    nc.scalar.activation(q_sb[:, ts(it, 512)], qkp[:D, :], IDENT, bias=bq_sb[:])
                    nc.scalar.activation(k_sb[:, ts(it, 512)], qkp[D:, :], IDENT, bias=bk_sb[:])

                    for jh in range(2):
                        vp = ps0.tile([128, 2, C], f32, tag="vp")
                        for jj in range(2):
                            j = jh * 2 + jj
                            for ko in range(KO):
                                nc.tensor.matmul(
                                    vp[:, jj, :], xb[:, ko, ts(j, 128)],
                                    wv_sb[:, ko, :],
                                    start=(ko == 0), stop=(ko == KO - 1),
                                )
                        vtt = vtio.tile([128, 2, C], fp8, tag="vtt")
                        nc.vector.tensor_tensor(
                            vtt[:], vp[:],
                            bv_sb[:, None, :].to_broadcast((128, 2, C)), ADD)
                        nc.gpsimd.dma_start(
                            vt_ap[ds(it * 512 + jh * 256, 256), :].rearrange(
                                "(jj p) c -> p jj c", p=128),
                            vtt[:]
                        )

            outp_cm = tc.tile_pool(name="outp", bufs=1)
            outp = outp_cm.__enter__()
            OUTB = outp.tile([128, KO, S], bf16)

            # column/row views of q, k: s = g*WP + w
            q_colv = q_sb[:, :].rearrange("d (g w) -> w d g", w=WP)
            k_colv = k_sb[:, :].rearrange("d (g w) -> w d g", w=WP)
            OUT_colv = OUTB[:, :, :].rearrange("p ko (g w) -> w p ko g", w=WP)

            # ---------------- Phases 1 & 2: attention ----------------
            with (
                tc.tile_pool(name="vtio2", bufs=vtio2_bufs) as vtio2,
                tc.tile_pool(name="attw", bufs=attw_bufs) as attw,
                tc.tile_pool(name="xpre", bufs=xpre_bufs) as xpre,
                tc.tile_pool(name="psA", bufs=psA_bufs, space="PSUM") as psA,
            ):
                # Phase 1: column (height-axis) attention, 4 columns/iter
                vt_col4 = vt_ap.rearrange("(g wq wr) c -> wq g wr c", wr=QB, g=HP)
                OUT_col4 = OUTB[:, :, :].rearrange(
                    "p ko (g wq wr) -> wq p ko g wr", wr=QB, g=HP
                )
                for wq in (range(WP // QB) if 1 in phases else []):
                    vtc = vtio2.tile([HP, QB, C], fp8, tag="vtc")
                    nc.gpsimd.dma_start(vtc[:], vt_col4[wq, :, :, :])
                    ep = psA.tile([HP, QB, HP], f32, tag="ep")
                    for r in range(QB):
                        w = wq * QB + r
                        nc.tensor.matmul(ep[:, r, :], k_colv[w, :, :],
                                         q_colv[w, :, :], start=True, stop=False)
                        nc.tensor.matmul(ep[:, r, :], negi_sb[:], eyeb_sb[:],
                                         start=False, stop=True)
                    ee = attw.tile([HP, QB, HP], bf16, tag="ee")
                    nc.scalar.activation(ee[:], ep[:], EXP, bias=shift_sb[:])
                    op = psA.tile([128, QB, 512], f32, tag="op")
                    for r in range(QB):
                        for cc in range(KO):
                            nc.tensor.matmul(op[:, r, ts(cc, HP)],
                                             vtc[:, r, ts(cc, 128)], ee[:, r, :],
                                             start=True, stop=True)
                    zp = psA.tile([HP, QB], f32, tag="zp")
                    for r in range(QB):
                        nc.tensor.matmul(zp[:, r:r + 1], ee[:, r, :], ones_sb[:],
                                         start=True, stop=True)
                    nc.scalar.copy(ZH[:, ts(wq, QB)], zp[:])
                    nc.vector.tensor_copy(
                        OUT_col4[wq, :, :, :, :],
                        op[:, :, :KO * HP].rearrange("p wr (ko g) -> p ko g wr", ko=KO))

                # Phase 2: row (width-axis) attention, 4 rows/iter,
                # split in halves; each half's normalization + final runs
                # while the next half computes.
                vt_row4 = vt_ap.rearrange("(hq hr t) c -> hq t hr c", hr=QB, t=HP)
                HALF = HP // 2
                NQH = HALF // QB

                def phase2_quad(hq):
                        vtr = vtio2.tile([HP, QB, C], fp8, tag="vtc")
                        nc.gpsimd.dma_start(vtr[:], vt_row4[hq, :, :, :])
                        ep2 = psA.tile([HP, QB, HP], f32, tag="ep")
                        for r in range(QB):
                            h = hq * QB + r
                            nc.tensor.matmul(ep2[:, r, :], k_sb[:, ds(h * WP, WP)],
                                             q_sb[:, ds(h * WP, WP)],
                                             start=True, stop=True)
                        ee2 = attw.tile([HP, QB, HP], bf16, tag="ee")
                        nc.scalar.activation(ee2[:], ep2[:], EXP, bias=shift_sb[:])
                        op2 = psA.tile([128, QB, 512], f32, tag="op")
                        for r in range(QB):
                            for cc in range(KO):
                                nc.tensor.matmul(op2[:, r, ts(cc, HP)],
                                                 vtr[:, r, ts(cc, 128)], ee2[:, r, :],
                                                 start=True, stop=True)
                        zp2 = psA.tile([HP, QB], f32, tag="zp")
                        for r in range(QB):
                            nc.tensor.matmul(zp2[:, r:r + 1], ee2[:, r, :], ones_sb[:],
                                             start=True, stop=True)
                        nc.scalar.copy(ZW[:, ts(hq, QB)], zp2[:])
                        outsl = OUTB[:, :, ds(hq * QB * WP, QB * WP)].rearrange(
                            "p ko (hr w) -> p hr ko w", hr=QB)
                        nc.vector.tensor_tensor(
                            outsl,
                            op2[:, :, :KO * HP].rearrange("p hr (ko w) -> p hr ko w", ko=KO),
                            outsl, ADD)

                def r_half(half):
                    # transposed orientation: [w parts, h-half free]
                    zs = consts.tile([HP, HALF], f32, tag=f"zs{half}")
                    nc.vector.tensor_tensor(zs[:], ZW[:, ds(half * HALF, HALF)],
                                            ZHT[:, ds(half * HALF, HALF)], ADD)
                    rm = consts.tile([HP, HALF], f32, tag=f"rm{half}")
                    nc.vector.reciprocal(rm[:], zs[:])
                    nc.vector.tensor_scalar_mul(rm[:], rm[:], gam_sb[:])
                    rmb = consts.tile([HP, HALF], bf16, tag=f"rmb{half}")
                    nc.vector.tensor_copy(rmb[:], rm[:])
                    # transposing DMA into h-major r_dram (2-byte elems, tiny)
                    nc.sync.dma_start(
                        r_ap[:, ds(half * HALF * WP, HALF * WP)].rearrange(
                            "a (h w) -> (a w) h", h=HALF), rmb[:])
                    nc.sync.dma_start(
                        rb[:, ds(half * HALF * WP, HALF * WP)],
                        r_ap[:, ds(half * HALF * WP, HALF * WP)].to_broadcast(
                            (128, HALF * WP)))

                xt2_tiles = {}

                def prefetch(it):
                    t = xpre.tile([128, KO, 512], f32, tag="xt2")
                    nc.gpsimd.dma_start(t[:], x_r[:, :, ts(it, 512)])
                    xt2_tiles[it] = t

                def final_tile(it, add_eng=None):
                    xt2 = xt2_tiles.pop(it)
                    t1 = fin.tile([128, KO, 512], f32, tag="t1")
                    nc.vector.tensor_tensor(
                        t1[:], OUTB[:, :, ts(it, 512)],
                        rb[:, None, ts(it, 512)].to_broadcast((128, KO, 512)),
                        MULT)
                    if add_eng is None:
                        add_eng = nc.vector if it % 2 == 0 else nc.gpsimd
                    add_eng.tensor_tensor(t1[:], t1[:], xt2[:], ADD)
                    nc.scalar.dma_start(out_r[:, :, ts(it, 512)], t1[:])

                if 2 in phases and 3 in phases:
                    zhtp = psA.tile([HP, HP], f32, tag="ep")
                    nc.tensor.transpose(zhtp[:], ZH[:], id_sb[:])
                    ZHT = consts.tile([HP, HP], f32)
                    nc.scalar.copy(ZHT[:], zhtp[:])
                    rb = consts.tile([128, S], bf16)
                    with tc.tile_pool(name="fin", bufs=fin_bufs) as fin:
                        # half 0 of phase 2, with x prefetch spread through it
                        for k, hq in enumerate(range(0, NQH)):
                            phase2_quad(hq)
                            if k % 6 == 5:
                                prefetch(k // 6)
                        r_half(0)
                        # half 1 of phase 2, interleaved with final half 0
                        for k, hq in enumerate(range(NQH, 2 * NQH)):
                            phase2_quad(hq)
                            if k % 3 == 2:
                                it = k // 3
                                final_tile(it, add_eng=nc.gpsimd)
                                if it + 3 < NT // 2 + 1:
                                    prefetch(it + 3)
                        final_tile(8)
                        r_half(1)
                        for it in range(NT // 2, NT):
                            if it + 1 < NT:
                                prefetch(it + 1) if it == NT // 2 else None
                            if it not in xt2_tiles:
                                prefetch(it)
                            final_tile(it)
                    qk_cm.__exit__(None, None, None)
                elif 2 in phases:
                    for hq in range(2 * NQH):
                        phase2_quad(hq)
                    qk_cm.__exit__(None, None, None)
                else:
                    qk_cm.__exit__(None, None, None)

            outp_cm.__exit__(None, None, None)

    nc.finalize()
    return nc
def _prep_in_maps(inputs):
    x = np.ascontiguousarray(np.asarray(inputs["x"]), dtype=np.float32)
    Wq = np.asarray(inputs["Wq"], dtype=np.float32)
    Wk = np.asarray(inputs["Wk"], dtype=np.float32)
    Wv = np.asarray(inputs["Wv"], dtype=np.float32)
    wqkT = np.ascontiguousarray(
        np.concatenate([Wq.T, Wk.T], axis=1)).astype(BF16).reshape(KO, 128, 2 * D)
    wvT = np.ascontiguousarray(Wv.T).astype(BF16).reshape(KO, 128, C)
    bq = np.asarray(inputs["bq"], dtype=np.float32).reshape(D, 1)
    bk = np.asarray(inputs["bk"], dtype=np.float32).reshape(D, 1)
    bv = np.asarray(inputs["bv"], dtype=np.float32).reshape(1, C)
    gamma = np.asarray(inputs["gamma"], dtype=np.float32).reshape(1, 1)
    id96 = np.eye(HP, dtype=np.float32)
    ones96 = np.ones((HP, 1), BF16)
    negeye = (np.eye(HP, dtype=np.float32) * np.float32(-1e30)).astype(BF16)
    eyeb = np.eye(HP, dtype=np.float32).astype(BF16)
    shared = dict(wqkT=wqkT, wvT=wvT, bq=bq, bk=bk, bv=bv,
                  gamma=gamma, id96=id96, ones96=ones96,
                  negeye=negeye, eyeb=eyeb)
    in_maps = []
    for i in range(N_CORES):
        m = dict(shared)
        m["x"] = np.ascontiguousarray(x[i].reshape(KO, 128, S))
        in_maps.append(m)
    return in_maps


def kernel(**inputs) -> np.ndarray:
    from concourse.bass_utils import run_bass_kernel_spmd

    if "nc" not in _cache:
        _cache["nc"] = _build_nc()
    nc = _cache["nc"]

    in_maps = _prep_in_maps(inputs)
    trace = bool(int(os.environ.get("CC_TRACE", "0")))
    res = run_bass_kernel_spmd(
        nc, in_maps, core_ids=list(range(N_CORES)), trace=trace
    )
    _cache["last_result"] = res
    out = np.stack(
        [np.asarray(res.results[i]["out"]).reshape(C, HP, WP) for i in range(N_CORES)]
    )
    return out
